# revision 12
# baseline (speedup 1.0000x reference)
"""Inverse Haar DWT2 (pywt 'haar' idwt2 convention) on 8 Trainium2 cores.

Input  x: [16, 256, 128, 128] f32 — 4 stacked subbands (LL|LH|HL|HH) of 64
channels each.  Output: [16, 64, 256, 256] f32.

Sharding: batch dim (16) split across 8 cores, 2 batches per core.  The
transform is elementwise per (batch, channel) — no communication.

Per-core kernel (x_loc [2, 256, 128, 128] -> y_loc [2, 64, 256, 256]):
SBUF partition dim = (batch, channel) = 2*64 = 128; tiles cover 16 input
rows (8/8 head for a fast ramp, 8/4/4 tail for a short drain).  Per tile:
  - 4 SWDGE DMAs (nc.gpsimd, one per subband) load Tb bf16 <- x f32 with
    the dtype cast done IN the SDMA datapath (SWDGE-only feature).  Band
    order in SBUF = (LL, HL, LH, HH).  Casting during the load halves the
    SBUF-side write bytes — the slow-HBM-regime runs trace to one SDMA
    engine (15) running its load queue ~25% slow and pacing the whole
    pipeline, and load descriptors' SBUF-write side is the only part of
    its byte count we can shrink (partition->port mapping is fixed and
    per-partition bytes are uniform, so rebalancing is impossible).
    bf16 roundoff (~2^-9) is far inside the 2e-2 gate.
  - prescale (ACT, in place, bf16): Tb[LL,LH] *= 0.5 — folds the first
    Haar 0.5; one strided (g, x) op over bands {0,2}.
  - stage 1 (DVE, bf16, fully contiguous halves -> 2x packed perf mode):
    UV[0:2fb] = Tb[0:2fb]+Tb[2fb:4fb] = (P|R),  P=(LL+LH)/2, R=HL+HH
    UV[2fb:4fb] = Tb[0:2fb]-Tb[2fb:4fb] = (Q|S), Q=(LL-LH)/2, S=HL-HH
  - stage 2 (DVE scalar_tensor_tensor, bf16 in / f32 out):
    out[2i+rr, 2j+s] = (R_or_S * +-0.5) + P_or_Q — the remaining Haar
    0.5 is folded into the stt scalar
  - 1 HWDGE DMA stores OUT f32, alternating between the SP and ACT
    HW-DGE rings per tile.  With loads on the SWDGE queue, neither HWDGE
    ring carries loads, so a store dma_start blocking its sequencer on
    the stage-2 semaphore gates nothing else, and consecutive (esp. the
    final two) stores issue/stream in parallel across the two rings.
Queues are fully decoupled: SWDGE = loads, SP/ACT = stores, Pool Q7 =
descriptor emission, ACT compute = prescale only (~1us/tile), DVE ~14us
per 16-row tile vs ~17us DMA cadence.

This container's walrus build supports only ONE semaphore wait per
instruction; Tile emits multi-wait instructions (incl. the final drain), so
after TileContext exit we redistribute extra waits onto single-wait NOPs
inserted before the instruction on the same engine.
"""

import numpy as np

import concourse.bass as bass
import concourse.mybir as mybir
from concourse.tile import TileContext
from concourse.bass_utils import run_bass_kernel_spmd

N_CORES = 8
B, C4, H, W = 16, 256, 128, 128
CH = C4 // 4          # 64 output channels
B_LOC = B // N_CORES  # 2 batches per core
F32 = mybir.dt.float32
BF16 = mybir.dt.bfloat16

# SBUF band order (LL, HL, LH, HH): Tb[0:2fb]+Tb[2fb:4fb] = (LL+LH | HL+HH)
# = (P|R) and the difference gives (Q|S) with fully contiguous APs.
BAND_SRC = (0, 2, 1, 3)  # Tb band t <- DRAM band BAND_SRC[t]


def _split_multi_waits(nc):
    """Move extra semaphore waits onto single-wait NOPs placed immediately
    before the over-subscribed instruction (same engine, so per-engine
    program order is preserved)."""
    n_split = 0
    for f in nc.m.functions:
        for blk in f.blocks:
            il = blk.instructions
            new_list = []
            for inst in il:
                si = getattr(inst, "sync_info", None)
                ow = si.on_wait if si is not None else None
                if ow and len(ow) > 1:
                    extra = list(ow[:-1])
                    del ow[:-1]
                    for w in extra:
                        n_split += 1
                        new_list.append(
                            mybir.InstNoOp(
                                name=f"{inst.name}-waitsplit-{n_split}",
                                engine=inst.engine,
                                sync_info=mybir.SyncInfo(on_wait=[w], on_update=[]),
                            )
                        )
                new_list.append(inst)
            il[:] = new_list
    return n_split


def _build_kernel():
    nc = bass.Bass("TRN2")
    x = nc.dram_tensor("x", [B_LOC, C4, H, W], F32, kind="ExternalInput")
    y = nc.dram_tensor("y", [B_LOC, CH, 2 * H, 2 * W], F32, kind="ExternalOutput")

    # 16-row tiles (8KB contiguous DRAM runs per partition per band);
    # 8/8 head so compute starts after 8 rows, 8/4/4 tail for short drain.
    tiles = (
        [(0, 8), (8, 8)]
        + [(16 + i * 16, 16) for i in range(6)]
        + [(112, 8), (120, 4), (124, 4)]
    )
    assert sum(hc for _, hc in tiles) == H

    with TileContext(nc) as tc:
        with (
            tc.tile_pool(name="tbf", bufs=3) as pbf,
            tc.tile_pool(name="tuv", bufs=2) as puv,
            tc.tile_pool(name="tout", bufs=3) as pout,
        ):
            for k, (h0, hc) in enumerate(tiles):
                fb = hc * W  # free elems per band block
                # ---- load with cast (SWDGE): Tb bf16 [p=(c,b)][band][i][w]
                # partition p = c*2 + b so the DRAM AP's outermost dim has
                # count 64 (the engine spray follows the outer source dim;
                # outer count 2 would use only 2 of 16 SDMA engines)
                Tb = pbf.tile([128, 4 * fb], BF16, tag="Tb")
                for tband in range(4):
                    sband = BAND_SRC[tband]
                    nc.gpsimd.dma_start(
                        out=Tb[:, tband * fb : (tband + 1) * fb],
                        in_=x[:, sband * CH : (sband + 1) * CH, h0 : h0 + hc, :]
                        .rearrange("b c h w -> c b (h w)"),
                    )
                # ---- prescale (ACT, in place): halve LL and LH = bands
                # {0,2} = (g, b2=0) plane of the (g, b2, x) band view.
                Tbg = Tb[:].rearrange("p (g b2 x) -> p g b2 x", g=2, b2=2)
                nc.scalar.mul(Tbg[:, :, 0], Tbg[:, :, 0], 0.5)
                # ---- stage 1 (DVE, bf16): butterfly over contiguous
                # halves.  1 free dim, unit stride, all-bf16 -> 2x packed
                # perf mode.
                UV = puv.tile([128, 4 * fb], BF16, tag="UV")  # [P|R|Q|S]
                nc.vector.tensor_add(
                    out=UV[:, : 2 * fb], in0=Tb[:, : 2 * fb], in1=Tb[:, 2 * fb :]
                )
                nc.vector.tensor_sub(
                    out=UV[:, 2 * fb :], in0=Tb[:, : 2 * fb], in1=Tb[:, 2 * fb :]
                )
                # ---- stage 2 (DVE): out[2i+rr, 2j+s] = P_or_Q + (-1)^s *
                # R_or_S / 2, computed as (R * +-0.5) + P with
                # scalar_tensor_tensor.  OUT free layout [i][rr][col],
                # col = 2j+s.  Keep every AP at <=2 free dims.
                OUT = pout.tile([128, 2 * hc * 2 * W], F32, tag="OUT")
                OUTv = OUT[:].rearrange(
                    "p (i r j s) -> p i r j s", i=hc, r=2, j=W, s=2
                )
                UVq = UV[:].rearrange("p (q i w) -> p q i w", q=4, i=hc)
                for rr in range(2):
                    P = UVq[:, 2 * rr]      # P (rr=0) or Q (rr=1), pre-halved
                    R = UVq[:, 2 * rr + 1]  # R (rr=0) or S (rr=1)
                    nc.vector.scalar_tensor_tensor(
                        out=OUTv[:, :, rr, :, 0], in0=R, scalar=0.5, in1=P,
                        op0=mybir.AluOpType.mult, op1=mybir.AluOpType.add,
                    )
                    nc.vector.scalar_tensor_tensor(
                        out=OUTv[:, :, rr, :, 1], in0=R, scalar=-0.5, in1=P,
                        op0=mybir.AluOpType.mult, op1=mybir.AluOpType.add,
                    )
                # ---- store (HWDGE, alternating SP/ACT rings; neither ring
                # carries anything else, so the sequencer-blocking store
                # wait gates nothing)
                eng = nc.sync if k % 2 == 0 else nc.scalar
                eng.dma_start(
                    out=y[:, :, 2 * h0 : 2 * h0 + 2 * hc, :]
                    .rearrange("b c h w -> c b (h w)"),
                    in_=OUT[:],
                )

    _split_multi_waits(nc)
    return nc


_NC_CACHE = None


def _get_nc():
    global _NC_CACHE
    if _NC_CACHE is None:
        _NC_CACHE = _build_kernel()
    return _NC_CACHE


def run_sharded(x, trace=False, **kwargs):
    assert x.shape == (B, C4, H, W) and x.dtype == np.float32
    nc = _get_nc()
    in_maps = [
        {"x": np.ascontiguousarray(x[i * B_LOC : (i + 1) * B_LOC])}
        for i in range(N_CORES)
    ]
    res = run_bass_kernel_spmd(
        nc, in_maps, core_ids=list(range(N_CORES)), trace=trace, **kwargs
    )
    out = np.concatenate([r["y"] for r in res.results], axis=0)
    return out, res


def kernel(x):
    out, _ = run_sharded(np.asarray(x))
    return out


# revision 13
# speedup vs baseline: 1.0536x; 1.0536x over previous
"""Inverse Haar DWT2 (pywt 'haar' idwt2 convention) on 8 Trainium2 cores.

Input  x: [16, 256, 128, 128] f32 — 4 stacked subbands (LL|LH|HL|HH) of 64
channels each.  Output: [16, 64, 256, 256] f32.

Sharding: batch dim (16) split across 8 cores, 2 batches per core.  The
transform is elementwise per (batch, channel) — no communication.

Per-core kernel (x_loc [2, 256, 128, 128] -> y_loc [2, 64, 256, 256]):
SBUF partition dim = (batch, channel) = 2*64 = 128.  Loads move 16 input
rows per DMA (8KB contiguous DRAM runs per partition per band — half the
per-descriptor overhead of 4KB runs); compute runs on 8-row sub-tiles.
  - 4 DMAs (one per subband) load T f32 on the SP HW-DGE ring, band order
    in SBUF = (LL, HL, LH, HH)
  - cast+prescale (ACT): Tb bf16 <- T * (0.5 for LL,LH | 1.0 for HL,HH).
    Halving LL,LH here folds the first Haar 0.5; bf16 intermediates make
    stage 1 eligible for the DVE 2x packed perf mode (needs all-2B dtypes
    and unit strides) and roundoff (~2^-9, measured 2.4e-3 norm rel err)
    is far inside the 2e-2 gate.
  - stage 1 (DVE, bf16, fully contiguous halves):
    UV[0:2fb] = Tb[0:2fb]+Tb[2fb:4fb] = (P|R),  P=(LL+LH)/2, R=HL+HH
    UV[2fb:4fb] = Tb[0:2fb]-Tb[2fb:4fb] = (Q|S), Q=(LL-LH)/2, S=HL-HH
  - stage 2 (DVE scalar_tensor_tensor, bf16 in / f32 out):
    out[2i+rr, 2j+s] = (R_or_S * +-0.5) + P_or_Q — the remaining Haar
    0.5 is folded into the stt scalar
  - 1 DMA stores OUT f32 on the ACT HW-DGE ring (separate ring from loads
    so a compute-gated store never head-of-line-blocks loads), deferred
    TWO sub-tiles: a store dma_start blocks the ACT sequencer on its
    stage-2 semaphore, so a one-iteration deferral serialized
    cast(k+1) behind stage2(k) (~11us/tile cycle > the 8.5us DMA cadence)
    and starved the SDMA engines; two iterations amortize the cycle over
    two tiles (~5.7us/tile, non-binding).
Engine budget per 8-row sub-tile: DVE ~6.9us, ACT ~4.5us, DMA ~8.5us
-> DMA-bound with comfortable DVE/ACT slack.  Measured (fast HBM regime):
~160us SDMA busy at ~410 GB/s aggregate (the HBM per-NC cap), <4us of
engine idle, ~9us fixed preamble+ramp -> ~173us.  Run-to-run HBM rate
varies ~25% on this part; the same binary measures 173-213us.

This container's walrus build supports only ONE semaphore wait per
instruction; Tile emits multi-wait instructions (incl. the final drain), so
after TileContext exit we redistribute extra waits onto single-wait NOPs
inserted before the instruction on the same engine.
"""

import numpy as np

import concourse.bass as bass
import concourse.mybir as mybir
from concourse.tile import TileContext
from concourse.bass_utils import run_bass_kernel_spmd

N_CORES = 8
B, C4, H, W = 16, 256, 128, 128
CH = C4 // 4          # 64 output channels
B_LOC = B // N_CORES  # 2 batches per core
HC = 8                # input rows per tile iteration
F32 = mybir.dt.float32
BF16 = mybir.dt.bfloat16

# SBUF band order (LL, HL, LH, HH): T[0:2fb]+T[2fb:4fb] = (LL+LH | HL+HH)
# = (2P | R) and the difference gives (2Q | S) with fully contiguous APs.
BAND_SRC = (0, 2, 1, 3)  # T band t <- DRAM band BAND_SRC[t]


def _split_multi_waits(nc):
    """Move extra semaphore waits onto single-wait NOPs placed immediately
    before the over-subscribed instruction (same engine, so per-engine
    program order is preserved)."""
    n_split = 0
    for f in nc.m.functions:
        for blk in f.blocks:
            il = blk.instructions
            new_list = []
            for inst in il:
                si = getattr(inst, "sync_info", None)
                ow = si.on_wait if si is not None else None
                if ow and len(ow) > 1:
                    extra = list(ow[:-1])
                    del ow[:-1]
                    for w in extra:
                        n_split += 1
                        new_list.append(
                            mybir.InstNoOp(
                                name=f"{inst.name}-waitsplit-{n_split}",
                                engine=inst.engine,
                                sync_info=mybir.SyncInfo(on_wait=[w], on_update=[]),
                            )
                        )
                new_list.append(inst)
            il[:] = new_list
    return n_split


def _build_kernel():
    nc = bass.Bass("TRN2")
    x = nc.dram_tensor("x", [B_LOC, C4, H, W], F32, kind="ExternalInput")
    y = nc.dram_tensor("y", [B_LOC, CH, 2 * H, 2 * W], F32, kind="ExternalOutput")

    # Load tiles are 16 rows (8KB contiguous DRAM runs per partition per
    # band -> half the descriptor count of 4KB runs; per-descriptor fixed
    # cost was ~15-30ns against ~140-180ns of streaming).  Compute tiles
    # stay 8 rows (two per load tile) so DVE/ACT slack and SBUF footprint
    # are unchanged.  Short taper so the post-last-load drain is cheap.
    load_tiles = (
        [(0, 8), (8, 8)]
        + [(16 + i * 16, 16) for i in range(6)]
        + [(112, 8), (120, 4), (124, 4)]
    )
    assert sum(hc for _, hc in load_tiles) == H

    with TileContext(nc) as tc:
        with (
            tc.tile_pool(name="tin", bufs=2) as pin,
            tc.tile_pool(name="tbf", bufs=3) as pbf,
            tc.tile_pool(name="tuv", bufs=3) as puv,
            tc.tile_pool(name="tout", bufs=4) as pout,
        ):
            # Stores are deferred TWO iterations.  A store dma_start on the
            # ACT queue blocks the sequencer until its stage-2 semaphore
            # fires, so with a one-iteration deferral cast(k+1) could not
            # start until stage2(k) finished — an ~11us/tile serial cycle
            # (cast 3.9 + stage1 2.4 + stage2 4.5) that outran the 8.5us
            # DMA cadence and starved the pipeline.  Two iterations of
            # deferral make the cycle span two tiles (~5.7us/tile amortized).
            pending_stores = []  # [(OUT tile, h0, hc), ...]

            def flush_store(engine=None):
                pOUT, ph0, phc = pending_stores.pop(0)
                (engine or nc.scalar).dma_start(
                    out=y[:, :, 2 * ph0 : 2 * ph0 + 2 * phc, :]
                    .rearrange("b c h w -> c b (h w)"),
                    in_=pOUT[:],
                )

            for h0, hcl in load_tiles:
                fbl = hcl * W  # free elems per band block of the load tile
                # ---- load: T [p=(c,b)][band][i][w]  (SP HW-DGE ring)
                # partition p = c*2 + b so the DRAM AP's outermost dim has
                # count 64 (the HWDGE engine spray follows the outer source
                # dim; outer count 2 would use only 2 of 16 SDMA engines)
                T = pin.tile([128, 4 * fbl], F32, tag="T")
                for tband in range(4):
                    sband = BAND_SRC[tband]
                    nc.sync.dma_start(
                        out=T[:, tband * fbl : (tband + 1) * fbl],
                        in_=x[:, sband * CH : (sband + 1) * CH, h0 : h0 + hcl, :]
                        .rearrange("b c h w -> c b (h w)"),
                    )
                Tv = T[:].rearrange(
                    "p (g b2 i w) -> p g b2 i w", g=2, b2=2, i=hcl
                )
                # ---- compute sub-tiles of 8 rows
                for hs in range(0, hcl, HC):
                    hc = min(HC, hcl - hs)
                    fb = hc * W
                    # ---- cast+prescale (ACT): Tb bf16 <- T * (0.5|1.0).
                    # Bands {0,2} (LL,LH) get the first Haar 0.5; bands
                    # {1,3} (HL,HH) are cast unscaled (their 0.5 rides the
                    # stage-2 stt scalar).  band = g*2 + b2 -> one ACT op
                    # per b2, reading the (g, i, w) view of the load tile,
                    # writing the contiguous (g, x) half of Tb.
                    Tb = pbf.tile([128, 4 * fb], BF16, tag="Tb")
                    Tbg = Tb[:].rearrange(
                        "p (g b2 i w) -> p g b2 i w", g=2, b2=2, i=hc
                    )
                    nc.scalar.mul(
                        Tbg[:, :, 0], Tv[:, :, 0, hs : hs + hc], 0.5
                    )
                    nc.scalar.mul(
                        Tbg[:, :, 1], Tv[:, :, 1, hs : hs + hc], 1.0
                    )
                    # ---- deferred store from TWO sub-tiles back
                    if len(pending_stores) >= 2:
                        flush_store()
                    # ---- stage 1 (DVE, bf16): butterfly over contiguous
                    # halves.  1 free dim, unit stride, all-bf16 -> 2x
                    # packed perf mode.
                    UV = puv.tile([128, 4 * fb], BF16, tag="UV")  # [P|R|Q|S]
                    nc.vector.tensor_add(
                        out=UV[:, : 2 * fb],
                        in0=Tb[:, : 2 * fb],
                        in1=Tb[:, 2 * fb :],
                    )
                    nc.vector.tensor_sub(
                        out=UV[:, 2 * fb :],
                        in0=Tb[:, : 2 * fb],
                        in1=Tb[:, 2 * fb :],
                    )
                    # ---- stage 2 (DVE): out[2i+rr, 2j+s] = P_or_Q +
                    # (-1)^s * R_or_S / 2, computed as (R * +-0.5) + P with
                    # scalar_tensor_tensor.  OUT free layout [i][rr][col],
                    # col = 2j+s.  Keep every AP at <=2 free dims.
                    OUT = pout.tile([128, 2 * hc * 2 * W], F32, tag="OUT")
                    OUTv = OUT[:].rearrange(
                        "p (i r j s) -> p i r j s", i=hc, r=2, j=W, s=2
                    )
                    UVq = UV[:].rearrange("p (q i w) -> p q i w", q=4, i=hc)
                    for rr in range(2):
                        P = UVq[:, 2 * rr]      # P (rr=0) or Q (rr=1)
                        R = UVq[:, 2 * rr + 1]  # R (rr=0) or S (rr=1)
                        nc.vector.scalar_tensor_tensor(
                            out=OUTv[:, :, rr, :, 0], in0=R, scalar=0.5,
                            in1=P,
                            op0=mybir.AluOpType.mult, op1=mybir.AluOpType.add,
                        )
                        nc.vector.scalar_tensor_tensor(
                            out=OUTv[:, :, rr, :, 1], in0=R, scalar=-0.5,
                            in1=P,
                            op0=mybir.AluOpType.mult, op1=mybir.AluOpType.add,
                        )
                    # ---- store (ACT HW-DGE ring, deferred two sub-tiles
                    # so stores never head-of-line-block loads or casts)
                    pending_stores.append((OUT, h0 + hs, hc))
            # flush the final deferred stores; the very last one goes on
            # the SP ring (loads are done by then) so the two tail stores
            # issue and stream in parallel instead of serializing on ACT.
            flush_store()
            flush_store(engine=nc.sync)

    _split_multi_waits(nc)
    return nc


_NC_CACHE = None


def _get_nc():
    global _NC_CACHE
    if _NC_CACHE is None:
        _NC_CACHE = _build_kernel()
    return _NC_CACHE


def run_sharded(x, trace=False, **kwargs):
    assert x.shape == (B, C4, H, W) and x.dtype == np.float32
    nc = _get_nc()
    in_maps = [
        {"x": np.ascontiguousarray(x[i * B_LOC : (i + 1) * B_LOC])}
        for i in range(N_CORES)
    ]
    res = run_bass_kernel_spmd(
        nc, in_maps, core_ids=list(range(N_CORES)), trace=trace, **kwargs
    )
    out = np.concatenate([r["y"] for r in res.results], axis=0)
    return out, res


def kernel(x):
    out, _ = run_sharded(np.asarray(x))
    return out
